# revision 1
# baseline (speedup 1.0000x reference)
"""Contextual attention kernel for Trainium2 (8 NeuronCores, data-parallel over batch).

Math (per batch b):
    Q = feaQK @ q_w.T + q_b
    k3 = conv1d(feaQK.T, cn3_w, SAME) + b3 ; k5 = conv1d(..., cn5_w) + b5
    K = [feaQK, k3, k5] @ k_w.T + k_b
    V = feaV @ v_w.T + v_b
    S = (Q @ K.T) / sqrt(D); mask keys >= seqlen with -inf
    out = softmax(S) @ V + V

Kernel strategy:
  * The convs + concat + K-projection collapse into a single width-5 stencil:
        K[s] = sum_{d=-2..2} feaQK[s+d] @ Wk[d] + kb_eff
    with Wk composed on the host (15 matmul-units of work -> 9).
  * All activations live on-chip in transposed layout ([feature, seq]) so no
    on-device transposes are needed anywhere:
        QT/KT from xT (host-transposed feaQK, zero-padded +-2 cols)
        scoresT[k,q] = KT chunks (stationary) x QT  (PSUM fp32)
        ET = exp(scoresT/32 + mask_bias[k])  (mask folded into exp bias; no
             max-subtraction needed since |scores/32| is O(1))
        V (natural [s,d]) from host-transposed feaV as the stationary operand
        outU[q,d] = ET chunks (stationary) x V; den[q] = ET x ones
        out = outU / den + V
  * Matmuls in bf16 (fp32 matmul is 4x slower on PE), fp32 PSUM accumulation.
  * Keys beyond seqlength are dead: K/scores/PV work only covers the first
    ceil(seqlen/128) key chunks per batch slot. Batches are paired
    longest-with-shortest across cores so the compile-time per-slot chunk
    counts (max over cores) stay small; sub-chunk masking still goes through
    the exp bias, so over-covering is always correct.
  * 16 batches -> 2 per core, full weights on every core.
"""

import numpy as np
import ml_dtypes

import concourse.bass as bass
from concourse import bacc
import concourse.tile as tile
from concourse import mybir

B, S, C, D = 16, 1024, 1024, 1024
P = 128
NCI, NDI, NKI, NQI, NSI = C // P, D // P, S // P, S // P, S // P
NF = 512  # matmul free dim (one PSUM bank of fp32)
PAD = 2
SP = S + 2 * PAD
LB = 2  # local batches per core
NCORES = 8
MASK_NEG = -60000.0
SCALE = 1.0 / 32.0  # 1/sqrt(D)

BF = mybir.dt.bfloat16
F32 = mybir.dt.float32
AF = mybir.ActivationFunctionType

TRACE = False  # set by test harness to collect HW profile
_CACHE = {}


def _build_program(vs):
    nc = bacc.Bacc("TRN2", dynamic_dma_scratch_size=256)

    xt = nc.dram_tensor("xt", [LB, C, SP], BF, kind="ExternalInput")
    fvt = nc.dram_tensor("fvt", [LB, C, S], BF, kind="ExternalInput")
    wq = nc.dram_tensor("wq", [C, D], BF, kind="ExternalInput")
    wk = nc.dram_tensor("wk", [5, C, D], BF, kind="ExternalInput")
    wv = nc.dram_tensor("wv", [C, D], BF, kind="ExternalInput")
    qb = nc.dram_tensor("qb", [P, NDI], F32, kind="ExternalInput")
    kb = nc.dram_tensor("kb", [P, NDI], F32, kind="ExternalInput")
    vb = nc.dram_tensor("vb", [P, D], F32, kind="ExternalInput")
    mb = nc.dram_tensor("mb", [LB, P, NKI], F32, kind="ExternalInput")
    out = nc.dram_tensor("out", [LB, S, D], F32, kind="ExternalOutput")

    with tile.TileContext(nc) as tc:
        _emit(nc, tc, xt, fvt, wq, wk, wv, qb, kb, vb, mb, out, vs)
    nc.finalize()
    return nc


def _emit(nc, tc, xt, fvt, wq, wk, wv, qb, kb, vb, mb, out, vs):
    from contextlib import ExitStack

    with ExitStack() as ctx:
        wpool = ctx.enter_context(tc.tile_pool(name="wpool", bufs=1))
        apool = ctx.enter_context(tc.tile_pool(name="apool", bufs=1))
        opool = ctx.enter_context(tc.tile_pool(name="opool", bufs=3))
        spool = ctx.enter_context(tc.tile_pool(name="spool", bufs=2))
        pp = ctx.enter_context(tc.tile_pool(name="pp", bufs=6, space="PSUM"))
        pd = ctx.enter_context(tc.tile_pool(name="pd", bufs=2, space="PSUM"))

        # Small constants first (cheap), then per-stage operands in the order
        # the PE consumes them, so the first matmul isn't stuck behind the
        # whole 18 MiB initial load (measured 51 us of PE idle).
        QB = wpool.tile([P, NDI], F32, tag="qb")
        nc.sync.dma_start(out=QB, in_=qb[:, :])
        KB = wpool.tile([P, NDI], F32, tag="kb")
        nc.sync.dma_start(out=KB, in_=kb[:, :])
        VB = wpool.tile([P, D], F32, tag="vb")
        nc.sync.dma_start(out=VB, in_=vb[:, :])
        ONES = wpool.tile([P, 1], BF, tag="ones")
        nc.vector.memset(ONES, 1.0)
        WV = wpool.tile([P, NCI, D], BF, tag="wv")
        WQ = wpool.tile([P, NCI, D], BF, tag="wq")
        WK = None

        for b in range(LB):
            v = vs[b]  # valid key chunks for this batch slot
            # key-dim psum groups: (offset, width) pieces covering v*128 cols
            kg = [(0, min(v * P, NF))]
            if v * P > NF:
                kg.append((NF, v * P - NF))

            # --- stage D: V natural [s, d] (first: smallest DMA prefix) --
            FVT = apool.tile([P, NCI, S], BF, tag="fvt")
            for ci in range(NCI):
                nc.sync.dma_start(out=FVT[:, ci, :], in_=fvt[b, ci * P:(ci + 1) * P, :])
                if b == 0:
                    nc.sync.dma_start(out=WV[:, ci, :], in_=wv[ci * P:(ci + 1) * P, :])
            V = apool.tile([P, NSI, D], BF, tag="v")
            for si in range(NSI):
                ps = [pp.tile([P, NF], F32, tag="ps", name=f"ps{_i}") for _i in range(2)]
                for ci in range(NCI):
                    lhsT = FVT[:, ci, si * P:(si + 1) * P]
                    for dh in range(2):
                        nc.tensor.matmul(
                            ps[dh], lhsT, WV[:, ci, dh * NF:(dh + 1) * NF],
                            start=(ci == 0), stop=(ci == NCI - 1))
                for dh in range(2):
                    nc.vector.tensor_add(
                        V[:, si, dh * NF:(dh + 1) * NF], ps[dh],
                        VB[:, dh * NF:(dh + 1) * NF])

            # --- stage B: QT[d, s] ---------------------------------------
            XT = apool.tile([P, NCI, SP], BF, tag="xt")
            for ci in range(NCI):
                nc.sync.dma_start(out=XT[:, ci, :], in_=xt[b, ci * P:(ci + 1) * P, :])
                if b == 0:
                    nc.sync.dma_start(out=WQ[:, ci, :], in_=wq[ci * P:(ci + 1) * P, :])
            MB = spool.tile([P, NKI], F32, tag="mb")
            nc.sync.dma_start(out=MB, in_=mb[b])
            QT = apool.tile([P, NDI, S], BF, tag="qt")
            for di in range(NDI):
                ps = [pp.tile([P, NF], F32, tag="ps", name=f"ps{_i}") for _i in range(2)]
                for ci in range(NCI):
                    lhsT = WQ[:, ci, di * P:(di + 1) * P]
                    for sh in range(2):
                        nc.tensor.matmul(
                            ps[sh], lhsT, XT[:, ci, PAD + sh * NF: PAD + sh * NF + NF],
                            start=(ci == 0), stop=(ci == NCI - 1))
                for sh in range(2):
                    nc.scalar.activation(
                        QT[:, di, sh * NF:(sh + 1) * NF], ps[sh], AF.Identity,
                        bias=QB[:, di:di + 1], scale=1.0)

            # --- stage C: KT[d, s] (width-5 stencil, only v key chunks) --
            if WK is None:
                WK = []
                for j in range(5):
                    t = wpool.tile([P, NCI, D], BF, tag=f"wk{j}")
                    for ci in range(NCI):
                        nc.sync.dma_start(
                            out=t[:, ci, :], in_=wk[j, ci * P:(ci + 1) * P, :])
                    WK.append(t)
            KT = apool.tile([P, NDI, S], BF, tag="kt")
            for di in range(NDI):
                ps = [pp.tile([P, NF], F32, tag="ps", name=f"ps{_i}")
                      for _i in range(len(kg))]
                step = 0
                for j in range(5):
                    for ci in range(NCI):
                        lhsT = WK[j][:, ci, di * P:(di + 1) * P]
                        for g, (off, w) in enumerate(kg):
                            nc.tensor.matmul(
                                ps[g][:, :w], lhsT,
                                XT[:, ci, j + off: j + off + w],
                                start=(step == 0), stop=(step == 5 * NCI - 1))
                        step += 1
                for g, (off, w) in enumerate(kg):
                    nc.scalar.activation(
                        KT[:, di, off:off + w], ps[g][:, :w], AF.Identity,
                        bias=KB[:, di:di + 1], scale=1.0)

            # --- stage E: ET[k, q] = exp(scoresT/32 + mask) --------------
            ET = apool.tile([P, NKI, S], BF, tag="et")
            for ki in range(v):
                ps = [pp.tile([P, NF], F32, tag="ps", name=f"ps{_i}") for _i in range(2)]
                for di in range(NDI):
                    lhsT = KT[:, di, ki * P:(ki + 1) * P]
                    for qh in range(2):
                        nc.tensor.matmul(
                            ps[qh], lhsT, QT[:, di, qh * NF:(qh + 1) * NF],
                            start=(di == 0), stop=(di == NDI - 1))
                for qh in range(2):
                    nc.scalar.activation(
                        ET[:, ki, qh * NF:(qh + 1) * NF], ps[qh], AF.Exp,
                        bias=MB[:, ki:ki + 1], scale=SCALE)

            # --- stage F: out = (ET^T @ V) / den + V ---------------------
            for qi in range(NQI):
                pso = [pp.tile([P, NF], F32, tag="ps", name=f"pso{_i}") for _i in range(2)]
                psd = pd.tile([P, 1], F32, tag="den")
                for ki in range(v):
                    lhsT = ET[:, ki, qi * P:(qi + 1) * P]
                    st, sp_ = (ki == 0), (ki == v - 1)
                    for dh in range(2):
                        nc.tensor.matmul(
                            pso[dh], lhsT, V[:, ki, dh * NF:(dh + 1) * NF],
                            start=st, stop=sp_)
                    nc.tensor.matmul(psd, lhsT, ONES, start=st, stop=sp_)
                # Free the PSUM banks with plain DVE copies that wait only on
                # the matmul stop; the reciprocal-scale and +V run in place on
                # SBUF afterwards, off the PE-critical path.
                OTs = []
                for dh in range(2):
                    OT = opool.tile([P, NF], F32, tag="out", name=f"ot{dh}")
                    nc.vector.tensor_copy(OT, pso[dh])
                    OTs.append(OT)
                REC = spool.tile([P, 1], F32, tag="rec")
                nc.vector.reciprocal(REC, psd)
                for dh in range(2):
                    OT = OTs[dh]
                    nc.scalar.activation(
                        OT, OT, AF.Copy, bias=0.0, scale=REC)
                    nc.vector.tensor_add(
                        OT, OT, V[:, qi, dh * NF:(dh + 1) * NF])
                    nc.sync.dma_start(
                        out=out[b, qi * P:(qi + 1) * P, dh * NF:(dh + 1) * NF],
                        in_=OT)


def _prep_host(feaQK, feaV, seqlengths, cn3_w, cn3_b, cn5_w, cn5_b,
               k_w, k_b, q_w, q_b, v_w, v_b):
    """Compose weights, assign batches to cores, lay out per-core inputs."""
    f32 = np.float32
    bf16 = ml_dtypes.bfloat16
    feaQK = np.asarray(feaQK, f32)
    feaV = np.asarray(feaV, f32)
    seqlengths = np.asarray(seqlengths).astype(np.int64)

    W1 = np.asarray(k_w, f32)[:, :C]
    W2 = np.asarray(k_w, f32)[:, C:2 * C]
    W3 = np.asarray(k_w, f32)[:, 2 * C:]

    wk = np.zeros((5, C, D), f32)  # [tap j (= shift+2), c, d]
    for t in range(3):
        wk[t + 1] += (W2 @ np.asarray(cn3_w, f32)[:, :, t]).T
    for t in range(5):
        wk[t] += (W3 @ np.asarray(cn5_w, f32)[:, :, t]).T
    wk[2] += W1.T
    kb_eff = (np.asarray(k_b, f32) + W2 @ np.asarray(cn3_b, f32)
              + W3 @ np.asarray(cn5_b, f32))

    wq = np.ascontiguousarray(np.asarray(q_w, f32).T)
    wv = np.ascontiguousarray(np.asarray(v_w, f32).T)

    qb_pd = np.ascontiguousarray(np.asarray(q_b, f32).reshape(NDI, P).T)
    kb_pd = np.ascontiguousarray(kb_eff.reshape(NDI, P).T)
    vb_rep = np.ascontiguousarray(
        np.broadcast_to(np.asarray(v_b, f32), (P, D)))

    key_valid = np.arange(S)[None, :] < seqlengths[:, None]
    mask = np.where(key_valid, 0.0, MASK_NEG).astype(f32)  # [B, S]

    # Pair longest with shortest so the compile-time per-slot chunk counts
    # (max over cores) stay near the per-core optimum.
    vchunks = np.clip(np.ceil(seqlengths / P).astype(int), 1, NKI)
    order = np.argsort(-seqlengths, kind="stable")
    batch_of = np.zeros((NCORES, LB), int)
    for i in range(NCORES):
        batch_of[i, 0] = order[B - 1 - i]
        batch_of[i, 1] = order[i]
    vs = (int(vchunks[batch_of[:, 0]].max()),
          int(vchunks[batch_of[:, 1]].max()))

    wq_b = wq.astype(bf16)
    wk_b = np.ascontiguousarray(wk.astype(bf16))
    wv_b = wv.astype(bf16)

    in_maps = []
    for core in range(NCORES):
        bs = batch_of[core]
        xts = np.zeros((LB, C, SP), bf16)
        xts[:, :, PAD:PAD + S] = feaQK[bs].transpose(0, 2, 1).astype(bf16)
        fvts = np.ascontiguousarray(
            feaV[bs].transpose(0, 2, 1)).astype(bf16)
        mbs = np.ascontiguousarray(
            mask[bs].reshape(LB, NKI, P).transpose(0, 2, 1))
        in_maps.append({
            "xt": xts, "fvt": fvts,
            "wq": wq_b, "wk": wk_b, "wv": wv_b,
            "qb": qb_pd, "kb": kb_pd, "vb": vb_rep, "mb": mbs,
        })
    return in_maps, batch_of, vs


def kernel(**inputs):
    from concourse.bass_utils import run_bass_kernel_spmd

    in_maps, batch_of, vs = _prep_host(**inputs)
    if _CACHE.get("vs") != vs:
        _CACHE["nc"] = _build_program(vs)
        _CACHE["vs"] = vs
    nc = _CACHE["nc"]
    res = run_bass_kernel_spmd(nc, in_maps, core_ids=list(range(NCORES)),
                               trace=TRACE)
    _CACHE["last_result"] = res
    full = np.zeros((B, S, D), np.float32)
    for core in range(NCORES):
        full[batch_of[core]] = res.results[core]["out"]
    return full



# revision 3
# speedup vs baseline: 1.4739x; 1.4739x over previous
"""Contextual attention kernel for Trainium2 (8 NeuronCores, data-parallel over batch).

Math (per batch b):
    Q = feaQK @ q_w.T + q_b
    k3 = conv1d(feaQK.T, cn3_w, SAME) + b3 ; k5 = conv1d(..., cn5_w) + b5
    K = [feaQK, k3, k5] @ k_w.T + k_b
    V = feaV @ v_w.T + v_b
    S = (Q @ K.T) / sqrt(D); mask keys >= seqlen with -inf
    out = softmax(S) @ V + V

Kernel strategy:
  * The convs + concat + K-projection collapse into a single width-5 stencil:
        K[s] = sum_{d=-2..2} feaQK[s+d] @ Wk[d] + kb_eff
    composed on the host (15 matmul-units of work -> 9).
  * All activations on-chip in transposed layout ([feature, seq]); no
    on-device transposes anywhere.
  * Q/K projections, scores, and PV run in fp8(e4m3) with
    perf_mode=DoubleRow: each matmul contracts 256 (2x128 chunk pairs) at
    ~2x bf16 ALU rate. Power-of-2 scale factors (exact in fp8) keep the
    tiny weights out of the subnormal range and every fp8-written tensor
    under the TRN e4m3 max of 240 (overflow would produce Inf, not
    saturation):
        x*2^4, wq/wk*2^12, QT/KT*2^5, ET*2^4 (folded into the exp bias as
        +4*ln2; cancels exactly against den = sum ET in the softmax ratio).
    Narrow psum groups (<256 cols) use plain fp8 matmuls (FWL beats
    DoubleRow's 256-col weight load there).
  * V projection stays bf16 (out ~= V + small attention average, so V's
    precision dominates the final error); V8 = fp8 copy of V feeds the PV
    moving operand.
  * Keys beyond seqlength are dead: K/scores/PV cover only the first
    ceil(seqlen/128) key chunks per batch slot; batches paired
    longest-with-shortest across cores keep the compile-time per-slot
    chunk counts small. Sub-chunk masking goes through the exp bias.
  * 16 batches -> 2 per core, full weights on every core.
"""

import numpy as np
import ml_dtypes

import concourse.bass as bass
from concourse import bacc
import concourse.tile as tile
from concourse import mybir

B, S, C, D = 16, 1024, 1024, 1024
P = 128
NCI, NDI, NKI, NQI, NSI = C // P, D // P, S // P, S // P, S // P
NF = 512  # matmul free dim (one PSUM bank of fp32)
PAD = 2
SP8 = 1040  # padded seq cols for fp8 x (mult of 16 for DR interleave APs)
LB = 2  # local batches per core
NCORES = 8
MASK_NEG = -60000.0
SCALE = 1.0 / 32.0  # 1/sqrt(D)

# fp8 power-of-2 scales (exact): see module docstring.
SX, SW, SQ, SK = 2.0**4, 2.0**12, 2.0**5, 2.0**5
EEXP = 4  # ET = 2^4 * exp(scores/32), via +EEXP*ln2 in the exp bias
QSCALE = SQ / (SX * SW)            # psum -> QT units
KSCALE = SK / (SX * SW)
ESCALE = SCALE / (SQ * SK)         # psum -> exp input
F8MAX = 240.0                      # TRN e4m3 max normal

BF = mybir.dt.bfloat16
F8 = mybir.dt.float8e4
F32 = mybir.dt.float32
AF = mybir.ActivationFunctionType
DR = mybir.MatmulPerfMode.DoubleRow
E4 = ml_dtypes.float8_e4m3

TRACE = False  # set by test harness to collect HW profile
_CACHE = {}


def _build_program(vs):
    nc = bacc.Bacc("TRN2", dynamic_dma_scratch_size=256)

    x8 = nc.dram_tensor("x8", [LB, C, SP8], F8, kind="ExternalInput")
    fvt = nc.dram_tensor("fvt", [LB, C, S], BF, kind="ExternalInput")
    wq8 = nc.dram_tensor("wq8", [C, D], F8, kind="ExternalInput")
    wk8 = nc.dram_tensor("wk8", [5, C, D], F8, kind="ExternalInput")
    wv = nc.dram_tensor("wv", [C, D], BF, kind="ExternalInput")
    qb = nc.dram_tensor("qb", [P, NDI], F32, kind="ExternalInput")
    kb = nc.dram_tensor("kb", [P, NDI], F32, kind="ExternalInput")
    vb = nc.dram_tensor("vb", [P, D], F32, kind="ExternalInput")
    mb = nc.dram_tensor("mb", [LB, P, NKI], F32, kind="ExternalInput")
    out = nc.dram_tensor("out", [LB, S, D], F32, kind="ExternalOutput")

    with tile.TileContext(nc) as tc:
        _emit(nc, tc, x8, fvt, wq8, wk8, wv, qb, kb, vb, mb, out, vs)
    nc.finalize()
    return nc


def _emit(nc, tc, x8, fvt, wq8, wk8, wv, qb, kb, vb, mb, out, vs):
    from contextlib import ExitStack

    with ExitStack() as ctx:
        wpool = ctx.enter_context(tc.tile_pool(name="wpool", bufs=1))
        apool = ctx.enter_context(tc.tile_pool(name="apool", bufs=1))
        opool = ctx.enter_context(tc.tile_pool(name="opool", bufs=3))
        spool = ctx.enter_context(tc.tile_pool(name="spool", bufs=2))
        pp = ctx.enter_context(tc.tile_pool(name="pp", bufs=6, space="PSUM"))
        pd = ctx.enter_context(tc.tile_pool(name="pd", bufs=2, space="PSUM"))

        QB = wpool.tile([P, NDI], F32, tag="qb")
        nc.sync.dma_start(out=QB, in_=qb[:, :])
        KB = wpool.tile([P, NDI], F32, tag="kb")
        nc.sync.dma_start(out=KB, in_=kb[:, :])
        VB = wpool.tile([P, D], F32, tag="vb")
        nc.sync.dma_start(out=VB, in_=vb[:, :])
        ONES = wpool.tile([P, 1], F8, tag="ones")
        nc.vector.memset(ONES, 1.0)
        WQ8 = wpool.tile([P, NCI, D], F8, tag="wq8")
        WV = wpool.tile([P, NCI, D], BF, tag="wv")
        WK8 = None

        for b in range(LB):
            v = vs[b]  # valid key chunks for this batch slot
            # key-dim psum groups: (offset, width) pieces covering v*128 cols
            kg = [(0, min(v * P, NF))]
            if v * P > NF:
                kg.append((NF, v * P - NF))

            # --- stage Q: QT8[d, s] (fp8 DoubleRow over ci pairs) --------
            X8 = apool.tile([P, NCI, SP8], F8, tag="x8")
            for ci in range(NCI):
                nc.sync.dma_start(out=X8[:, ci, :], in_=x8[b, ci * P:(ci + 1) * P, :])
                if b == 0:
                    nc.sync.dma_start(out=WQ8[:, ci, :], in_=wq8[ci * P:(ci + 1) * P, :])
            MB = spool.tile([P, NKI], F32, tag="mb")
            nc.sync.dma_start(out=MB, in_=mb[b])
            QT8 = apool.tile([P, NDI, S], F8, tag="qt8")
            for di in range(NDI):
                ps = [pp.tile([P, NF], F32, tag="ps", name=f"ps{_i}") for _i in range(2)]
                for c0 in range(0, NCI, 2):
                    lhsT = WQ8[:, c0:c0 + 2, di * P:(di + 1) * P]
                    for sh in range(2):
                        nc.tensor.matmul(
                            ps[sh], lhsT,
                            X8[:, c0:c0 + 2, PAD + sh * NF: PAD + sh * NF + NF],
                            start=(c0 == 0), stop=(c0 == NCI - 2), perf_mode=DR)
                for sh in range(2):
                    nc.scalar.activation(
                        QT8[:, di, sh * NF:(sh + 1) * NF], ps[sh], AF.Identity,
                        bias=QB[:, di:di + 1], scale=QSCALE)

            # --- stage K: KT8[d, s] (width-5 stencil, v key chunks) ------
            if WK8 is None:
                WK8 = []
                for j in range(5):
                    t = wpool.tile([P, NCI, D], F8, tag=f"wk8{j}")
                    for ci in range(NCI):
                        nc.sync.dma_start(
                            out=t[:, ci, :], in_=wk8[j, ci * P:(ci + 1) * P, :])
                    WK8.append(t)
            KT8 = apool.tile([P, NDI, S], F8, tag="kt8")
            for di in range(NDI):
                ps = [pp.tile([P, NF], F32, tag="ps", name=f"ps{_i}")
                      for _i in range(len(kg))]
                # per-group matmul counters for start/stop bookkeeping
                ndr = [w >= 256 for (_, w) in kg]
                total = [(5 * NCI // 2) if d else 5 * NCI for d in ndr]
                done = [0] * len(kg)
                for j in range(5):
                    for c0 in range(0, NCI, 2):
                        for g, (off, w) in enumerate(kg):
                            if ndr[g]:
                                nc.tensor.matmul(
                                    ps[g][:, :w],
                                    WK8[j][:, c0:c0 + 2, di * P:(di + 1) * P],
                                    X8[:, c0:c0 + 2, j + off: j + off + w],
                                    start=(done[g] == 0),
                                    stop=(done[g] == total[g] - 1),
                                    perf_mode=DR)
                                done[g] += 1
                            else:
                                for cc in (c0, c0 + 1):
                                    nc.tensor.matmul(
                                        ps[g][:, :w],
                                        WK8[j][:, cc, di * P:(di + 1) * P],
                                        X8[:, cc, j + off: j + off + w],
                                        start=(done[g] == 0),
                                        stop=(done[g] == total[g] - 1))
                                    done[g] += 1
                for g, (off, w) in enumerate(kg):
                    nc.scalar.activation(
                        KT8[:, di, off:off + w], ps[g][:, :w], AF.Identity,
                        bias=KB[:, di:di + 1], scale=KSCALE)

            # --- stage E: ET8[k, q] = 2^4 exp(scoresT/32 + mask) ---------
            ET8 = apool.tile([P, NKI, S], F8, tag="et8")
            for ki in range(v):
                ps = [pp.tile([P, NF], F32, tag="ps", name=f"ps{_i}") for _i in range(2)]
                for d0 in range(0, NDI, 2):
                    lhsT = KT8[:, d0:d0 + 2, ki * P:(ki + 1) * P]
                    for qh in range(2):
                        nc.tensor.matmul(
                            ps[qh], lhsT, QT8[:, d0:d0 + 2, qh * NF:(qh + 1) * NF],
                            start=(d0 == 0), stop=(d0 == NDI - 2), perf_mode=DR)
                for qh in range(2):
                    nc.scalar.activation(
                        ET8[:, ki, qh * NF:(qh + 1) * NF], ps[qh], AF.Exp,
                        bias=MB[:, ki:ki + 1], scale=ESCALE)

            # --- stage V: V natural [s, d] (bf16) + fp8 copy for PV ------
            FVT = apool.tile([P, NCI, S], BF, tag="fvt")
            for ci in range(NCI):
                nc.sync.dma_start(out=FVT[:, ci, :], in_=fvt[b, ci * P:(ci + 1) * P, :])
                if b == 0:
                    nc.sync.dma_start(out=WV[:, ci, :], in_=wv[ci * P:(ci + 1) * P, :])
            V = apool.tile([P, NSI, D], BF, tag="v")
            V8 = apool.tile([P, NKI, D], F8, tag="v8")
            for si in range(NSI):
                ps = [pp.tile([P, NF], F32, tag="ps", name=f"ps{_i}") for _i in range(2)]
                for ci in range(NCI):
                    lhsT = FVT[:, ci, si * P:(si + 1) * P]
                    for dh in range(2):
                        nc.tensor.matmul(
                            ps[dh], lhsT, WV[:, ci, dh * NF:(dh + 1) * NF],
                            start=(ci == 0), stop=(ci == NCI - 1))
                for dh in range(2):
                    nc.vector.tensor_add(
                        V[:, si, dh * NF:(dh + 1) * NF], ps[dh],
                        VB[:, dh * NF:(dh + 1) * NF])
                if si < v:
                    for dh in range(2):
                        nc.scalar.activation(
                            V8[:, si, dh * NF:(dh + 1) * NF],
                            V[:, si, dh * NF:(dh + 1) * NF],
                            AF.Copy, bias=0.0, scale=1.0)

            # --- stage F: out = (ET^T @ V) / den + V ---------------------
            for qi in range(NQI):
                pso = [pp.tile([P, NF], F32, tag="ps", name=f"pso{_i}") for _i in range(2)]
                psd = pd.tile([P, 1], F32, tag="den")
                vev = v - (v % 2)
                for k0 in range(0, vev, 2):
                    lhsT = ET8[:, k0:k0 + 2, qi * P:(qi + 1) * P]
                    st, sp_ = (k0 == 0), (k0 + 2 >= v)
                    for dh in range(2):
                        nc.tensor.matmul(
                            pso[dh], lhsT, V8[:, k0:k0 + 2, dh * NF:(dh + 1) * NF],
                            start=st, stop=sp_, perf_mode=DR)
                if v % 2:
                    lhsT = ET8[:, v - 1, qi * P:(qi + 1) * P]
                    for dh in range(2):
                        nc.tensor.matmul(
                            pso[dh], lhsT, V8[:, v - 1, dh * NF:(dh + 1) * NF],
                            start=(v == 1), stop=True)
                for ki in range(v):
                    nc.tensor.matmul(
                        psd, ET8[:, ki, qi * P:(qi + 1) * P], ONES,
                        start=(ki == 0), stop=(ki == v - 1))
                # Free the PSUM banks with plain DVE copies that wait only on
                # the matmul stop; the reciprocal-scale and +V run in place on
                # SBUF afterwards, off the PE-critical path.
                OTs = []
                for dh in range(2):
                    OT = opool.tile([P, NF], F32, tag="out", name=f"ot{dh}")
                    nc.vector.tensor_copy(OT, pso[dh])
                    OTs.append(OT)
                REC = spool.tile([P, 1], F32, tag="rec")
                nc.vector.reciprocal(REC, psd)
                for dh in range(2):
                    OT = OTs[dh]
                    nc.scalar.activation(
                        OT, OT, AF.Copy, bias=0.0, scale=REC)
                    nc.vector.tensor_add(
                        OT, OT, V[:, qi, dh * NF:(dh + 1) * NF])
                    nc.sync.dma_start(
                        out=out[b, qi * P:(qi + 1) * P, dh * NF:(dh + 1) * NF],
                        in_=OT)


def _prep_host(feaQK, feaV, seqlengths, cn3_w, cn3_b, cn5_w, cn5_b,
               k_w, k_b, q_w, q_b, v_w, v_b):
    """Compose weights, assign batches to cores, lay out per-core inputs."""
    f32 = np.float32
    bf16 = ml_dtypes.bfloat16
    feaQK = np.asarray(feaQK, f32)
    feaV = np.asarray(feaV, f32)
    seqlengths = np.asarray(seqlengths).astype(np.int64)

    W1 = np.asarray(k_w, f32)[:, :C]
    W2 = np.asarray(k_w, f32)[:, C:2 * C]
    W3 = np.asarray(k_w, f32)[:, 2 * C:]

    wk = np.zeros((5, C, D), f32)  # [tap j (= shift+2), c, d]
    for t in range(3):
        wk[t + 1] += (W2 @ np.asarray(cn3_w, f32)[:, :, t]).T
    for t in range(5):
        wk[t] += (W3 @ np.asarray(cn5_w, f32)[:, :, t]).T
    wk[2] += W1.T
    kb_eff = (np.asarray(k_b, f32) + W2 @ np.asarray(cn3_b, f32)
              + W3 @ np.asarray(cn5_b, f32))

    def q8(a, s):
        return np.clip(np.asarray(a, f32) * s, -F8MAX, F8MAX).astype(E4)

    wq8 = np.ascontiguousarray(q8(np.asarray(q_w, f32).T, SW))
    wk8 = np.ascontiguousarray(q8(wk, SW))
    wv_b = np.ascontiguousarray(np.asarray(v_w, f32).T).astype(bf16)

    qb_pd = np.ascontiguousarray(
        (np.asarray(q_b, f32) * SQ).reshape(NDI, P).T)
    kb_pd = np.ascontiguousarray((kb_eff * SK).reshape(NDI, P).T)
    vb_rep = np.ascontiguousarray(
        np.broadcast_to(np.asarray(v_b, f32), (P, D)))

    key_valid = np.arange(S)[None, :] < seqlengths[:, None]
    mask = np.where(key_valid, EEXP * np.log(2.0), MASK_NEG).astype(f32)

    # Pair longest with shortest so the compile-time per-slot chunk counts
    # (max over cores) stay near the per-core optimum.
    vchunks = np.clip(np.ceil(seqlengths / P).astype(int), 1, NKI)
    order = np.argsort(-seqlengths, kind="stable")
    batch_of = np.zeros((NCORES, LB), int)
    for i in range(NCORES):
        batch_of[i, 0] = order[B - 1 - i]
        batch_of[i, 1] = order[i]
    vs = (int(vchunks[batch_of[:, 0]].max()),
          int(vchunks[batch_of[:, 1]].max()))

    in_maps = []
    for core in range(NCORES):
        bs = batch_of[core]
        x8s = np.zeros((LB, C, SP8), E4)
        x8s[:, :, PAD:PAD + S] = q8(feaQK[bs].transpose(0, 2, 1), SX)
        fvts = np.ascontiguousarray(
            feaV[bs].transpose(0, 2, 1)).astype(bf16)
        mbs = np.ascontiguousarray(
            mask[bs].reshape(LB, NKI, P).transpose(0, 2, 1))
        in_maps.append({
            "x8": x8s, "fvt": fvts,
            "wq8": wq8, "wk8": wk8, "wv": wv_b,
            "qb": qb_pd, "kb": kb_pd, "vb": vb_rep, "mb": mbs,
        })
    return in_maps, batch_of, vs


def kernel(**inputs):
    from concourse.bass_utils import run_bass_kernel_spmd

    in_maps, batch_of, vs = _prep_host(**inputs)
    if _CACHE.get("vs") != vs:
        _CACHE["nc"] = _build_program(vs)
        _CACHE["vs"] = vs
    nc = _CACHE["nc"]
    res = run_bass_kernel_spmd(nc, in_maps, core_ids=list(range(NCORES)),
                               trace=TRACE)
    _CACHE["last_result"] = res
    full = np.zeros((B, S, D), np.float32)
    for core in range(NCORES):
        full[batch_of[core]] = res.results[core]["out"]
    return full


# revision 8
# speedup vs baseline: 1.5243x; 1.0342x over previous
"""Contextual attention kernel for Trainium2 (8 NeuronCores, data-parallel over batch).

Math (per batch b):
    Q = feaQK @ q_w.T + q_b
    k3 = conv1d(feaQK.T, cn3_w, SAME) + b3 ; k5 = conv1d(..., cn5_w) + b5
    K = [feaQK, k3, k5] @ k_w.T + k_b
    V = feaV @ v_w.T + v_b
    S = (Q @ K.T) / sqrt(D); mask keys >= seqlen with -inf
    out = softmax(S) @ V + V

Kernel strategy:
  * The convs + concat + K-projection collapse into a single width-5 stencil:
        K[s] = sum_{d=-2..2} feaQK[s+d] @ Wk[d] + kb_eff
    composed on the host (15 matmul-units of work -> 9).
  * All activations on-chip in transposed layout ([feature, seq]); no
    on-device transposes anywhere.
  * Q/K projections, scores, and PV run in fp8(e4m3) with
    perf_mode=DoubleRow: each matmul contracts 256 (2x128 chunk pairs) at
    ~2x bf16 ALU rate. Power-of-2 scale factors (exact in fp8) keep the
    tiny weights out of the subnormal range and every fp8-written tensor
    under the TRN e4m3 max of 240 (overflow would produce Inf, not
    saturation):
        x*2^4, wq/wk*2^12, QT/KT*2^5, ET*2^4 (folded into the exp bias as
        +4*ln2; cancels exactly against den = sum ET in the softmax ratio).
    Narrow psum groups (<256 cols) use plain fp8 matmuls (FWL beats
    DoubleRow's 256-col weight load there).
  * V projection stays bf16 (out ~= V + small attention average, so V's
    precision dominates the final error); V8 = fp8 copy of V feeds the PV
    moving operand.
  * Keys beyond seqlength are dead: K/scores/PV cover only the first
    ceil(seqlen/128) key chunks per batch slot; batches paired
    longest-with-shortest across cores keep the compile-time per-slot
    chunk counts small. Sub-chunk masking goes through the exp bias.
  * 16 batches -> 2 per core, full weights on every core.
"""

import numpy as np
import ml_dtypes

import concourse.bass as bass
from concourse import bacc
import concourse.tile as tile
from concourse import mybir

B, S, C, D = 16, 1024, 1024, 1024
P = 128
NCI, NDI, NKI, NQI, NSI = C // P, D // P, S // P, S // P, S // P
NF = 512  # matmul free dim (one PSUM bank of fp32)
PAD = 2
SP8 = 1040  # padded seq cols for fp8 x (mult of 16 for DR interleave APs)
LB = 2  # local batches per core
NCORES = 8
MASK_NEG = -60000.0
SCALE = 1.0 / 32.0  # 1/sqrt(D)

# fp8 power-of-2 scales (exact): see module docstring.
SX, SW, SQ, SK = 2.0**4, 2.0**12, 2.0**5, 2.0**5
EEXP = 4  # ET = 2^4 * exp(scores/32), via +EEXP*ln2 in the exp bias
QSCALE = SQ / (SX * SW)            # psum -> QT units
KSCALE = SK / (SX * SW)
ESCALE = SCALE / (SQ * SK)         # psum -> exp input
F8MAX = 240.0                      # TRN e4m3 max normal

BF = mybir.dt.bfloat16
F8 = mybir.dt.float8e4
F32 = mybir.dt.float32
AF = mybir.ActivationFunctionType
DR = mybir.MatmulPerfMode.DoubleRow
E4 = ml_dtypes.float8_e4m3

TRACE = False  # set by test harness to collect HW profile
_CACHE = {}


def _build_program(vs):
    nc = bacc.Bacc("TRN2", dynamic_dma_scratch_size=256)

    x8 = nc.dram_tensor("x8", [LB, C, SP8], F8, kind="ExternalInput")
    fvt = nc.dram_tensor("fvt", [LB, C, S], BF, kind="ExternalInput")
    wq8 = nc.dram_tensor("wq8", [C, D], F8, kind="ExternalInput")
    wk8 = nc.dram_tensor("wk8", [5, C, D], F8, kind="ExternalInput")
    wv = nc.dram_tensor("wv", [C, D], BF, kind="ExternalInput")
    qb = nc.dram_tensor("qb", [P, NDI], F32, kind="ExternalInput")
    kb = nc.dram_tensor("kb", [P, NDI], F32, kind="ExternalInput")
    vb = nc.dram_tensor("vb", [P, D], F32, kind="ExternalInput")
    mb = nc.dram_tensor("mb", [LB, P, NKI], F32, kind="ExternalInput")
    out = nc.dram_tensor("out", [LB, S, D], BF, kind="ExternalOutput")

    with tile.TileContext(nc) as tc:
        _emit(nc, tc, x8, fvt, wq8, wk8, wv, qb, kb, vb, mb, out, vs)
    nc.finalize()
    return nc


def _emit(nc, tc, x8, fvt, wq8, wk8, wv, qb, kb, vb, mb, out, vs):
    from contextlib import ExitStack

    with ExitStack() as ctx:
        wpool = ctx.enter_context(tc.tile_pool(name="wpool", bufs=1))
        apool = ctx.enter_context(tc.tile_pool(name="apool", bufs=1))
        opool = ctx.enter_context(tc.tile_pool(name="opool", bufs=3))
        spool = ctx.enter_context(tc.tile_pool(name="spool", bufs=2))
        pp = ctx.enter_context(tc.tile_pool(name="pp", bufs=6, space="PSUM"))
        pd = ctx.enter_context(tc.tile_pool(name="pd", bufs=2, space="PSUM"))

        QB = wpool.tile([P, NDI], F32, tag="qb")
        nc.sync.dma_start(out=QB, in_=qb[:, :])
        KB = wpool.tile([P, NDI], F32, tag="kb")
        nc.sync.dma_start(out=KB, in_=kb[:, :])
        VB = wpool.tile([P, D], F32, tag="vb")
        nc.sync.dma_start(out=VB, in_=vb[:, :])
        ONES = wpool.tile([P, 1], F8, tag="ones")
        nc.vector.memset(ONES, 1.0)
        WQ8 = wpool.tile([P, NCI, D], F8, tag="wq8")
        WV = wpool.tile([P, NCI, D], BF, tag="wv")
        WK8 = None

        for b in range(LB):
            v = vs[b]  # valid key chunks for this batch slot
            # key-dim psum groups: (offset, width) pieces covering v*128 cols
            kg = [(0, min(v * P, NF))]
            if v * P > NF:
                kg.append((NF, v * P - NF))

            # --- stage Q: QT8[d, s] (fp8 DoubleRow over ci pairs) --------
            X8 = apool.tile([P, NCI, SP8], F8, tag="x8")
            for ci in range(NCI):
                nc.sync.dma_start(out=X8[:, ci, :], in_=x8[b, ci * P:(ci + 1) * P, :])
                if b == 0:
                    # low di columns first so early psum groups can start
                    # before the whole 1 MiB of wq8 lands
                    nc.sync.dma_start(
                        out=WQ8[:, ci, :NF],
                        in_=wq8[ci * P:(ci + 1) * P, :NF])
            if b == 0:
                for ci in range(NCI):
                    nc.sync.dma_start(
                        out=WQ8[:, ci, NF:],
                        in_=wq8[ci * P:(ci + 1) * P, NF:])
            MB = spool.tile([P, NKI], F32, tag="mb")
            nc.sync.dma_start(out=MB, in_=mb[b])
            QT8 = apool.tile([P, NDI, S], F8, tag="qt8")
            for di in range(NDI):
                ps = [pp.tile([P, NF], F32, tag="ps", name=f"ps{_i}") for _i in range(2)]
                for c0 in range(0, NCI, 2):
                    lhsT = WQ8[:, c0:c0 + 2, di * P:(di + 1) * P]
                    for sh in range(2):
                        nc.tensor.matmul(
                            ps[sh], lhsT,
                            X8[:, c0:c0 + 2, PAD + sh * NF: PAD + sh * NF + NF],
                            start=(c0 == 0), stop=(c0 == NCI - 2), perf_mode=DR)
                for sh in range(2):
                    nc.scalar.activation(
                        QT8[:, di, sh * NF:(sh + 1) * NF], ps[sh], AF.Identity,
                        bias=QB[:, di:di + 1], scale=QSCALE)

            # --- stage K: KT8[d, s] (width-5 stencil, v key chunks) ------
            if WK8 is None:
                WK8 = []
                for j in range(5):
                    t = wpool.tile([P, NCI, D], F8, tag=f"wk8{j}")
                    for ci in range(NCI):
                        nc.sync.dma_start(
                            out=t[:, ci, :], in_=wk8[j, ci * P:(ci + 1) * P, :])
                    WK8.append(t)
            KT8 = apool.tile([P, NDI, S], F8, tag="kt8")
            for di in range(NDI):
                ps = [pp.tile([P, NF], F32, tag="ps", name=f"ps{_i}")
                      for _i in range(len(kg))]
                # per-group matmul counters for start/stop bookkeeping
                ndr = [w >= 256 for (_, w) in kg]
                total = [(5 * NCI // 2) if d else 5 * NCI for d in ndr]
                done = [0] * len(kg)
                for j in range(5):
                    for c0 in range(0, NCI, 2):
                        for g, (off, w) in enumerate(kg):
                            if ndr[g]:
                                nc.tensor.matmul(
                                    ps[g][:, :w],
                                    WK8[j][:, c0:c0 + 2, di * P:(di + 1) * P],
                                    X8[:, c0:c0 + 2, j + off: j + off + w],
                                    start=(done[g] == 0),
                                    stop=(done[g] == total[g] - 1),
                                    perf_mode=DR)
                                done[g] += 1
                            else:
                                for cc in (c0, c0 + 1):
                                    nc.tensor.matmul(
                                        ps[g][:, :w],
                                        WK8[j][:, cc, di * P:(di + 1) * P],
                                        X8[:, cc, j + off: j + off + w],
                                        start=(done[g] == 0),
                                        stop=(done[g] == total[g] - 1))
                                    done[g] += 1
                for g, (off, w) in enumerate(kg):
                    nc.scalar.activation(
                        KT8[:, di, off:off + w], ps[g][:, :w], AF.Identity,
                        bias=KB[:, di:di + 1], scale=KSCALE)

            # --- stage E: ET8[k, q] = 2^4 exp(scoresT/32 + mask) ---------
            ET8 = apool.tile([P, NKI, S], F8, tag="et8")
            for ki in range(v):
                ps = [pp.tile([P, NF], F32, tag="ps", name=f"ps{_i}") for _i in range(2)]
                for d0 in range(0, NDI, 2):
                    lhsT = KT8[:, d0:d0 + 2, ki * P:(ki + 1) * P]
                    for qh in range(2):
                        nc.tensor.matmul(
                            ps[qh], lhsT, QT8[:, d0:d0 + 2, qh * NF:(qh + 1) * NF],
                            start=(d0 == 0), stop=(d0 == NDI - 2), perf_mode=DR)
                for qh in range(2):
                    nc.scalar.activation(
                        ET8[:, ki, qh * NF:(qh + 1) * NF], ps[qh], AF.Exp,
                        bias=MB[:, ki:ki + 1], scale=ESCALE)

            # --- stage V: V natural [s, d] (bf16) + fp8 copy for PV ------
            FVT = apool.tile([P, NCI, S], BF, tag="fvt")
            for ci in range(NCI):
                nc.sync.dma_start(out=FVT[:, ci, :], in_=fvt[b, ci * P:(ci + 1) * P, :])
                if b == 0:
                    nc.sync.dma_start(out=WV[:, ci, :], in_=wv[ci * P:(ci + 1) * P, :])
            V = apool.tile([P, NSI, D], BF, tag="v")
            V8 = apool.tile([P, NKI, D], F8, tag="v8")
            for si in range(NSI):
                ps = [pp.tile([P, NF], F32, tag="ps", name=f"ps{_i}") for _i in range(2)]
                for ci in range(NCI):
                    lhsT = FVT[:, ci, si * P:(si + 1) * P]
                    for dh in range(2):
                        nc.tensor.matmul(
                            ps[dh], lhsT, WV[:, ci, dh * NF:(dh + 1) * NF],
                            start=(ci == 0), stop=(ci == NCI - 1))
                for dh in range(2):
                    nc.vector.tensor_add(
                        V[:, si, dh * NF:(dh + 1) * NF], ps[dh],
                        VB[:, dh * NF:(dh + 1) * NF])
                if si < v:
                    for dh in range(2):
                        nc.scalar.activation(
                            V8[:, si, dh * NF:(dh + 1) * NF],
                            V[:, si, dh * NF:(dh + 1) * NF],
                            AF.Copy, bias=0.0, scale=1.0)

            # --- stage F: out = (ET^T @ V) / den + V ---------------------
            for qi in range(NQI):
                pso = [pp.tile([P, NF], F32, tag="ps", name=f"pso{_i}") for _i in range(2)]
                psd = pd.tile([P, 1], F32, tag="den")
                vev = v - (v % 2)
                for k0 in range(0, vev, 2):
                    lhsT = ET8[:, k0:k0 + 2, qi * P:(qi + 1) * P]
                    st, sp_ = (k0 == 0), (k0 + 2 >= v)
                    for dh in range(2):
                        nc.tensor.matmul(
                            pso[dh], lhsT, V8[:, k0:k0 + 2, dh * NF:(dh + 1) * NF],
                            start=st, stop=sp_, perf_mode=DR)
                if v % 2:
                    lhsT = ET8[:, v - 1, qi * P:(qi + 1) * P]
                    for dh in range(2):
                        nc.tensor.matmul(
                            pso[dh], lhsT, V8[:, v - 1, dh * NF:(dh + 1) * NF],
                            start=(v == 1), stop=True)
                for ki in range(v):
                    nc.tensor.matmul(
                        psd, ET8[:, ki, qi * P:(qi + 1) * P], ONES,
                        start=(ki == 0), stop=(ki == v - 1))
                # Free the PSUM banks with plain DVE copies that wait only on
                # the matmul stop; the reciprocal-scale and +V run in place on
                # SBUF afterwards, off the PE-critical path.
                OTs = []
                for dh in range(2):
                    OT = opool.tile([P, NF], F32, tag="out", name=f"ot{dh}")
                    nc.vector.tensor_copy(OT, pso[dh])
                    OTs.append(OT)
                REC = spool.tile([P, 1], F32, tag="rec")
                nc.vector.reciprocal(REC, psd)
                for dh in range(2):
                    OT = OTs[dh]
                    nc.scalar.activation(
                        OT, OT, AF.Copy, bias=0.0, scale=REC)
                    OB = opool.tile([P, NF], BF, tag="outb", name=f"ob{dh}")
                    nc.vector.tensor_add(
                        OB, OT, V[:, qi, dh * NF:(dh + 1) * NF])
                    nc.sync.dma_start(
                        out=out[b, qi * P:(qi + 1) * P, dh * NF:(dh + 1) * NF],
                        in_=OB)


def _prep_host(feaQK, feaV, seqlengths, cn3_w, cn3_b, cn5_w, cn5_b,
               k_w, k_b, q_w, q_b, v_w, v_b):
    """Compose weights, assign batches to cores, lay out per-core inputs."""
    f32 = np.float32
    bf16 = ml_dtypes.bfloat16
    feaQK = np.asarray(feaQK, f32)
    feaV = np.asarray(feaV, f32)
    seqlengths = np.asarray(seqlengths).astype(np.int64)

    W1 = np.asarray(k_w, f32)[:, :C]
    W2 = np.asarray(k_w, f32)[:, C:2 * C]
    W3 = np.asarray(k_w, f32)[:, 2 * C:]

    wk = np.zeros((5, C, D), f32)  # [tap j (= shift+2), c, d]
    for t in range(3):
        wk[t + 1] += (W2 @ np.asarray(cn3_w, f32)[:, :, t]).T
    for t in range(5):
        wk[t] += (W3 @ np.asarray(cn5_w, f32)[:, :, t]).T
    wk[2] += W1.T
    kb_eff = (np.asarray(k_b, f32) + W2 @ np.asarray(cn3_b, f32)
              + W3 @ np.asarray(cn5_b, f32))

    def q8(a, s):
        return np.clip(np.asarray(a, f32) * s, -F8MAX, F8MAX).astype(E4)

    wq8 = np.ascontiguousarray(q8(np.asarray(q_w, f32).T, SW))
    wk8 = np.ascontiguousarray(q8(wk, SW))
    wv_b = np.ascontiguousarray(np.asarray(v_w, f32).T).astype(bf16)

    qb_pd = np.ascontiguousarray(
        (np.asarray(q_b, f32) * SQ).reshape(NDI, P).T)
    kb_pd = np.ascontiguousarray((kb_eff * SK).reshape(NDI, P).T)
    vb_rep = np.ascontiguousarray(
        np.broadcast_to(np.asarray(v_b, f32), (P, D)))

    key_valid = np.arange(S)[None, :] < seqlengths[:, None]
    mask = np.where(key_valid, EEXP * np.log(2.0), MASK_NEG).astype(f32)

    # Pair longest with shortest so the compile-time per-slot chunk counts
    # (max over cores) stay near the per-core optimum.
    vchunks = np.clip(np.ceil(seqlengths / P).astype(int), 1, NKI)
    order = np.argsort(-seqlengths, kind="stable")
    batch_of = np.zeros((NCORES, LB), int)
    for i in range(NCORES):
        batch_of[i, 0] = order[B - 1 - i]
        batch_of[i, 1] = order[i]
    vs = (int(vchunks[batch_of[:, 0]].max()),
          int(vchunks[batch_of[:, 1]].max()))

    in_maps = []
    for core in range(NCORES):
        bs = batch_of[core]
        x8s = np.zeros((LB, C, SP8), E4)
        x8s[:, :, PAD:PAD + S] = q8(feaQK[bs].transpose(0, 2, 1), SX)
        fvts = np.ascontiguousarray(
            feaV[bs].transpose(0, 2, 1)).astype(bf16)
        mbs = np.ascontiguousarray(
            mask[bs].reshape(LB, NKI, P).transpose(0, 2, 1))
        in_maps.append({
            "x8": x8s, "fvt": fvts,
            "wq8": wq8, "wk8": wk8, "wv": wv_b,
            "qb": qb_pd, "kb": kb_pd, "vb": vb_rep, "mb": mbs,
        })
    return in_maps, batch_of, vs


def kernel(**inputs):
    from concourse.bass_utils import run_bass_kernel_spmd

    in_maps, batch_of, vs = _prep_host(**inputs)
    if _CACHE.get("vs") != vs:
        _CACHE["nc"] = _build_program(vs)
        _CACHE["vs"] = vs
    nc = _CACHE["nc"]
    res = run_bass_kernel_spmd(nc, in_maps, core_ids=list(range(NCORES)),
                               trace=TRACE)
    _CACHE["last_result"] = res
    full = np.zeros((B, S, D), np.float32)
    for core in range(NCORES):
        full[batch_of[core]] = res.results[core]["out"].astype(np.float32)
    return full
